# revision 42
# baseline (speedup 1.0000x reference)
"""Trainium2 Bass kernel for a pre-norm transformer block (nn_Block_38843684225792).

Full inputs -> full outputs. Sharding: data-parallel over batch, one batch
element per NeuronCore (8 cores). Inside each core the block is computed
channel-major (channels on SBUF partitions) so every matmul contracts over
the partition dim; x is transposed once on entry and the result transposed
back on exit via PE transposes.

v2: all weights DMA'd directly from their natural DRAM layout (no
DRAM->DRAM reorg), biases folded into PSUM accumulation via K=1 outer
products, LayerNorm scale/shift broadcast via K=1 PE matmuls + DVE apply
(no scalar-engine ACTs on the LN path), softmax denominators normalized
with a 64-row gpsimd broadcast, fc2 uses f32r stationary weights with bf16
moving activations (no weight casts), and output transposes stream out in
bf16 interleaved with the fc2 chains.

Shapes (per core): x [1024, 768], heads=12, hd=64, mlp hidden=3072.
"""

import os
import sys

sys.path.insert(0, "/opt/trn_rl_repo")

import numpy as np

import concourse.bass as bass
import concourse.tile as tile
from concourse import bacc, mybir
from concourse.bass_utils import run_bass_kernel_spmd
from concourse.masks import make_identity

F32 = mybir.dt.float32
F32R = mybir.dt.float32r
BF16 = mybir.dt.bfloat16
AF = mybir.ActivationFunctionType
ALU = mybir.AluOpType

N_CORES = 8
S = 1024          # sequence length per core
C = 768           # model dim
H = 12            # heads
HD = 64           # head dim
HID = 3072        # mlp hidden
NCH = C // 128    # 6 channel chunks
NT = S // 128     # 8 token chunks
NFH = HID // 128  # 24 hidden chunks
EPS = 1e-5
ATT_SCALE = HD ** -0.5  # 0.125

_cached = {}


def build():
    nc = bacc.Bacc(None, target_bir_lowering=False, debug=False)
    x_d = nc.declare_dram_parameter("x", [S, C], F32, isOutput=False)
    ln1_g_d = nc.declare_dram_parameter("ln1_g", [C], F32, isOutput=False)
    ln1_b_d = nc.declare_dram_parameter("ln1_b", [C], F32, isOutput=False)
    w_qkv_d = nc.declare_dram_parameter("w_qkv", [C, 3 * C], F32, isOutput=False)
    w_proj_d = nc.declare_dram_parameter("w_proj", [C, C], F32, isOutput=False)
    b_proj_d = nc.declare_dram_parameter("b_proj", [C], F32, isOutput=False)
    ln2_g_d = nc.declare_dram_parameter("ln2_g", [C], F32, isOutput=False)
    ln2_b_d = nc.declare_dram_parameter("ln2_b", [C], F32, isOutput=False)
    w_fc1_d = nc.declare_dram_parameter("w_fc1", [C, HID], F32, isOutput=False)
    b_fc1_d = nc.declare_dram_parameter("b_fc1", [HID], F32, isOutput=False)
    w_fc2_d = nc.declare_dram_parameter("w_fc2", [HID, C], F32, isOutput=False)
    b_fc2_d = nc.declare_dram_parameter("b_fc2", [C], F32, isOutput=False)
    out_d = nc.declare_dram_parameter("out", [S, C], F32, isOutput=True)
    dbg = bool(int(os.environ.get("BASS_DEBUG_TAPS", "0")))
    if dbg:
        dbg_h = nc.declare_dram_parameter("dbg_h", [C, S], F32, isOutput=True)
        dbg_attn = nc.declare_dram_parameter("dbg_attn", [C, S], F32, isOutput=True)
        dbg_out1 = nc.declare_dram_parameter("dbg_out1", [C, S], F32, isOutput=True)
        dbg_h2 = nc.declare_dram_parameter("dbg_h2", [C, S], F32, isOutput=True)

    from contextlib import ExitStack
    with tile.TileContext(nc) as tc, ExitStack() as ctx:
        consts = ctx.enter_context(tc.tile_pool(name="consts", bufs=1))
        arena = ctx.enter_context(tc.tile_pool(name="arena", bufs=1))
        work = ctx.enter_context(tc.tile_pool(name="work", bufs=1))

        # ---------------- constants ----------------
        ident = consts.tile([128, 128], F32, name="ident")
        make_identity(nc, ident)
        ident_bf = consts.tile([128, 128], BF16, name="ident_bf")
        nc.vector.tensor_copy(out=ident_bf, in_=ident)
        ones_row = consts.tile([1, 512], F32, name="ones_row")
        nc.vector.memset(ones_row, 1.0)
        ones_row_r = consts.tile([1, 512], F32R, name="ones_row_r")
        nc.vector.tensor_copy(out=ones_row_r, in_=ones_row)
        eps_ap = consts.tile([1, 1], F32, name="eps_ap")
        nc.vector.memset(eps_ap, EPS)
        eps_col = consts.tile([128, 1], F32, name="eps_col")
        nc.vector.memset(eps_col, EPS)
        ones_f32 = consts.tile([128, 1], F32, name="ones_f32")
        nc.vector.memset(ones_f32, 1.0)
        ones_col = consts.tile([128, 1], F32R, name="ones_col")
        nc.vector.tensor_copy(out=ones_col, in_=ones_f32)  # stat sums
        ones_r128 = consts.tile([1, 128], F32, name="ones_r128")
        nc.vector.memset(ones_r128, 1.0)
        ones_col_row = consts.tile([1, 128], F32R, name="ones_col_row")
        nc.vector.tensor_copy(out=ones_col_row, in_=ones_r128)  # K=1 bcasts

        # per-channel vectors, [128, n] column layout
        def load_chanvec(dram_t, name, width):
            t = consts.tile([128, width], F32, name=name)
            nc.sync.dma_start(out=t, in_=dram_t.ap().rearrange("(o p) -> p o", p=128))
            return t

        g1c = load_chanvec(ln1_g_d, "g1c", NCH)
        b1c = load_chanvec(ln1_b_d, "b1c", NCH)
        g2c = load_chanvec(ln2_g_d, "g2c", NCH)
        b2c = load_chanvec(ln2_b_d, "b2c", NCH)
        bf1 = load_chanvec(b_fc1_d, "bf1", NFH)

        # bias row vectors [1, C] for K=1 outer-product bias folds
        def load_rowvec(dram_t, name, width):
            t = consts.tile([1, width], F32R, name=name)
            nc.sync.dma_start(
                out=t,
                in_=dram_t.ap().rearrange("(a c) -> a c", a=1).bitcast(F32R))
            return t

        bpc = load_chanvec(b_proj_d, "bpc", NCH)
        bf2c = load_chanvec(b_fc2_d, "bf2c", NCH)

        # ---------------- persistent activation tiles ----------------
        xT = [arena.tile([128, S], F32R, tag=f"xT{c}", name=f"xT{c}")
              for c in range(NCH)]
        hT = [arena.tile([128, S], F32R, tag=f"hT{c}", name=f"hT{c}")
              for c in range(NCH)]
        attnT = [arena.tile([128, S], F32R, tag=f"attnT{c}", name=f"attnT{c}")
                 for c in range(NCH)]
        out1T = [arena.tile([128, S], F32R, tag=f"out1T{c}", name=f"out1T{c}")
                 for c in range(NCH)]

        # ================= LayerNorm helper pieces =================
        def ln_stats_half(ps, src_tiles, n, tagp):
            # psum2[1,0:512] = sum(x), [1,512:1024] = sum(x^2), one token half
            sl = slice(512 * n, 512 * (n + 1))
            psum2 = ps.tile([1, S], F32, tag="stat", bufs=1,
                            name=f"ln_s{tagp}{n}")
            for c in range(NCH):
                x2 = work.tile([128, 512], F32R, tag="mid", bufs=3,
                               name=f"ln_x2_{tagp}{n}{c}")
                nc.vector.tensor_mul(out=x2, in0=src_tiles[c].bitcast(F32)[:, sl],
                                     in1=src_tiles[c].bitcast(F32)[:, sl])
                nc.tensor.matmul(psum2[:, 0:512], ones_col, src_tiles[c][:, sl],
                                 start=(c == 0), stop=(c == NCH - 1))
                nc.tensor.matmul(psum2[:, 512:1024], ones_col, x2,
                                 start=(c == 0), stop=(c == NCH - 1))
            return psum2

        def ln_chain_half(psum2, n, tagp):
            mu = work.tile([1, 512], F32, tag="stats", bufs=4,
                           name=f"ln_mu{tagp}{n}")
            nc.scalar.mul(out=mu, in_=psum2[:, 0:512], mul=1.0 / C)
            ex2 = work.tile([1, 512], F32, tag="stats", bufs=4,
                            name=f"ln_ex2{tagp}{n}")
            nc.scalar.mul(out=ex2, in_=psum2[:, 512:1024], mul=1.0 / C)
            var = work.tile([1, 512], F32, tag="stats", bufs=4,
                            name=f"ln_var{tagp}{n}")
            nc.vector.tensor_mul(out=var, in0=mu, in1=mu)
            nc.vector.tensor_sub(out=var, in0=ex2, in1=var)
            nc.scalar.activation(out=var, in_=var, func=AF.Ln, bias=eps_ap,
                                 scale=1.0)
            rstd = work.tile([1, 512], F32R, tag="stats", bufs=4,
                             name=f"ln_rstd{tagp}{n}")
            nc.scalar.activation(out=rstd, in_=var, func=AF.Exp, bias=0.0,
                                 scale=-0.5)
            negmuR = work.tile([1, 512], F32R, tag="stats", bufs=4,
                               name=f"ln_nmu{tagp}{n}")
            nc.vector.scalar_tensor_tensor(out=negmuR, in0=mu, scalar=-1.0,
                                           in1=rstd.bitcast(F32), op0=ALU.mult,
                                           op1=ALU.mult)
            return rstd, negmuR

        def ln_bcast_half(ps, rstd, negmuR, tagp, n, tag="bc", bufs=2):
            # A0 = ones ⊗ rstd, B0 = ones ⊗ (-mu*rstd)   (both [128, 512])
            A0 = ps.tile([128, 512], F32, tag=tag, bufs=bufs,
                         name=f"ln_A{tagp}{n}")
            nc.tensor.matmul(A0, ones_col_row, rstd.bitcast(F32R))
            B0 = ps.tile([128, 512], F32, tag=tag, bufs=bufs,
                         name=f"ln_B{tagp}{n}")
            nc.tensor.matmul(B0, ones_col_row, negmuR.bitcast(F32R))
            return A0, B0

        def ln_apply_half(src_tiles, dst_tiles, A0, B0, gc, bc, n, tagp):
            sl = slice(512 * n, 512 * (n + 1))
            # SBUF copies of the broadcasts so chunks 4-5 can run on gpsimd
            # (which cannot read PSUM) in parallel with the DVE chunks.
            A0s = work.tile([128, 512], F32, tag="gtmp", bufs=2,
                            name=f"ln_A0s{tagp}{n}")
            nc.vector.tensor_copy(out=A0s, in_=A0)
            B0s = work.tile([128, 512], F32, tag="gtmp", bufs=2,
                            name=f"ln_B0s{tagp}{n}")
            nc.vector.tensor_copy(out=B0s, in_=B0)
            for c in range(NCH):
                eng = nc.gpsimd if c >= 4 else nc.vector
                a0, b0 = (A0s, B0s) if c >= 4 else (A0, B0)
                tg = "gtmp2" if c >= 4 else "tmp"
                t = work.tile([128, 512], F32, tag=tg, bufs=2,
                              name=f"ln_t{tagp}{n}{c}")
                eng.tensor_mul(out=t, in0=src_tiles[c].bitcast(F32)[:, sl],
                               in1=a0)
                eng.tensor_add(out=t, in0=t, in1=b0)
                eng.tensor_scalar(
                    out=dst_tiles[c][:, sl], in0=t,
                    scalar1=gc[:, c:c + 1], scalar2=bc[:, c:c + 1],
                    op0=ALU.mult, op1=ALU.add)

        # ---------------- attention-side weights pool ----------------
        with tc.tile_pool(name="wvqk", bufs=1) as wvqk:
            def dma_qk(p):
                tiles = []
                for which, m in (("q", p), ("k", 6 + p)):
                    wqk = wvqk.tile([128, NCH, 128], F32R, tag=f"w{which}",
                                    bufs=2, name=f"w{which}{p}")
                    nc.scalar.dma_start(
                        out=wqk,
                        in_=w_qkv_d.ap()[:, 128 * m:128 * (m + 1)]
                        .rearrange("(ko ki) m -> ki ko m", ki=128)
                        .bitcast(F32R))
                    tiles.append(wqk)
                return tiles

            # v_aug[p, mt, head, 66]: [v(64), one, pad]; the ones column makes
            # the O matmul emit softmax denominators at psum partition 64.
            v_aug = wvqk.tile([128, NT, H, 66], BF16, tag="v_aug", name="v_aug")
            nc.vector.memset(v_aug[:, :, :, 64:65], 1.0)

            # ------- entry: v/qk weight DMAs, transposes, LN1, v stage ------
            with tc.tile_pool(name="wvp", bufs=1) as wvp:
                with tc.tile_pool(name="ps_e", bufs=1, space="PSUM") as ps_e:
                    # token-major LN1 stat columns: sums/sumsq per token
                    sumsT = work.tile([128, NT], F32, tag="stats", bufs=4,
                                      name="l1_sumsT")
                    sqT = work.tile([128, NT], F32, tag="stats", bufs=4,
                                    name="l1_sqT")

                    def transpose_in(a):
                        # 6-way DMA split: one 512B-per-partition strip per
                        # queue so x streams in at full aggregate bandwidth.
                        x_sb = work.tile([128, C], F32, tag="bigst", bufs=2,
                                         name=f"x_sb{a}")
                        for c in range(NCH):
                            nc.sync.dma_start(
                                out=x_sb[:, 128 * c:128 * (c + 1)],
                                in_=x_d.ap()[128 * a:128 * (a + 1),
                                             128 * c:128 * (c + 1)])
                        # free-dim reductions on the scalar engine give this
                        # chunk's LN1 sums without touching the PE
                        jnk = work.tile([128, C], BF16, tag="actjunk", bufs=2,
                                        name=f"actjunk{a}")
                        nc.scalar.activation(out=jnk, in_=x_sb, func=AF.Copy,
                                             accum_out=sumsT[:, a:a + 1])
                        nc.scalar.activation(out=jnk, in_=x_sb, func=AF.Square,
                                             accum_out=sqT[:, a:a + 1])
                        for c in range(NCH):
                            pst = ps_e.tile([128, 128], F32, tag="small",
                                            bufs=3, name=f"ptx{a}_{c}")
                            nc.tensor.transpose(
                                pst, x_sb[:, 128 * c:128 * (c + 1)], ident)
                            nc.vector.tensor_copy(
                                out=xT[c][:, 128 * a:128 * (a + 1)], in_=pst)

                    # spin the PE for ~5us so HAM unthrottles to full
                    # clock before the first real transposes arrive.
                    warm = consts.tile([128, 128], F32R, name="warm")
                    nc.vector.tensor_copy(out=warm, in_=ident)
                    jps = ps_e.tile([1, 128], F32, tag="small", bufs=3,
                                    name="warm_ps")
                    for _ in range(45):
                        nc.tensor.matmul(jps, ones_col, warm)

                    transpose_in(0)
                    def spin(n):
                        for _ in range(n):
                            nc.tensor.matmul(jps, ones_col, warm)
                    spin(20)
                    transpose_in(1)
                    spin(20)
                    # weight streams queue behind the first two x chunks
                    wv = []
                    for i in range(12):
                        n, ko = i // NCH, i % NCH
                        w = wvp.tile([128, 384], F32R, tag=f"wv{i}", bufs=1,
                                     name=f"wv{n}_{ko}")
                        nc.scalar.dma_start(
                            out=w,
                            in_=w_qkv_d.ap()[128 * ko:128 * (ko + 1),
                                             1536 + 384 * n:1536 + 384 * (n + 1)]
                            .bitcast(F32R))
                        wv.append(w)
                    qk_w = {0: dma_qk(0), 1: dma_qk(1)}

                    def v_stage(mts):
                        for nn in range(2):  # halves of the 768 v-channels
                            for mt in mts:
                                pv = ps_e.tile([128, 384], F32, tag="small",
                                               bufs=3, name=f"pv{nn}_{mt}")
                                for ko in range(NCH):
                                    nc.tensor.matmul(
                                        pv, hT[ko][:, 128 * mt:128 * (mt + 1)],
                                        wv[nn * NCH + ko],
                                        start=(ko == 0), stop=(ko == NCH - 1))
                                pv3 = pv.rearrange("p (j d) -> p j d", d=HD)
                                nc.vector.tensor_copy(
                                    out=v_aug[:, mt, 6 * nn:6 * nn + 6, 0:64],
                                    in_=pv3)

                    transpose_in(2)
                    spin(20)
                    transpose_in(3)
                    spin(20)
                    for a in range(4, 8):
                        transpose_in(a)
                    # token-major chain: mu, var, rstd, -mu*rstd  ([128, NT])
                    muT = work.tile([128, NT], F32, tag="stats", bufs=4,
                                    name="l1_muT")
                    nc.scalar.mul(out=muT, in_=sumsT, mul=1.0 / C)
                    varT = work.tile([128, NT], F32, tag="stats", bufs=4,
                                     name="l1_varT")
                    nc.vector.tensor_mul(out=varT, in0=muT, in1=muT)
                    nc.vector.scalar_tensor_tensor(
                        out=varT, in0=sqT, scalar=1.0 / C, in1=varT,
                        op0=ALU.mult, op1=ALU.subtract)
                    nc.scalar.activation(out=varT, in_=varT, func=AF.Ln,
                                         bias=eps_col, scale=1.0)
                    rstdT = work.tile([128, NT], F32, tag="stats", bufs=4,
                                      name="l1_rstdT")
                    nc.scalar.activation(out=rstdT, in_=varT, func=AF.Exp,
                                         bias=0.0, scale=-0.5)
                    nmrT = work.tile([128, NT], F32, tag="stats", bufs=4,
                                     name="l1_nmrT")
                    nc.vector.scalar_tensor_tensor(
                        out=nmrT, in0=muT, scalar=-1.0, in1=rstdT,
                        op0=ALU.mult, op1=ALU.mult)
                    # pack [128, NT] token-major stats into [1, S] rows via a
                    # PE transpose + an SBUF-to-SBUF flattening DMA.
                    rows = {}
                    for nm, colT in (("rstd", rstdT), ("nmr", nmrT)):
                        pt = ps_e.tile([NT, 128], F32, tag="small", bufs=3,
                                       name=f"l1_{nm}_pt")
                        nc.tensor.transpose(pt, colT, ident)
                        r8 = work.tile([NT, 128], F32, tag="tmp", bufs=2,
                                       name=f"l1_{nm}_r8")
                        nc.vector.tensor_copy(out=r8, in_=pt)
                        row = work.tile([1, S], F32R, tag="mid", bufs=3,
                                        name=f"l1_{nm}_row")
                        nc.sync.dma_start(out=row.bitcast(F32), in_=r8)
                        rows[nm] = row
                    A00, B00 = ln_bcast_half(
                        ps_e, rows["rstd"].bitcast(F32)[:, 0:512],
                        rows["nmr"].bitcast(F32)[:, 0:512], "l1", 0)
                    ln_apply_half(xT, hT, A00, B00, g1c, b1c, 0, "l1")
                    spin(60)
                    A01, B01 = ln_bcast_half(
                        ps_e, rows["rstd"].bitcast(F32)[:, 512:1024],
                        rows["nmr"].bitcast(F32)[:, 512:1024], "l1", 1)
                    v_stage(range(0, 4))
                    ln_apply_half(xT, hT, A01, B01, g1c, b1c, 1, "l1")
                    spin(25)
                    v_stage(range(4, 8))

            if dbg:
                for c in range(NCH):
                    nc.sync.dma_start(
                        out=dbg_h.ap()[128 * c:128 * (c + 1), :],
                        in_=hT[c].bitcast(F32))

            # ---------------- attention ----------------
            wp_tiles = []
            es_wp = ExitStack()
            wpp = es_wp.enter_context(
                tc.tile_pool(name="wpp", bufs=1, side="right"))
            with tc.tile_pool(name="ps_a", bufs=1, space="PSUM") as ps_a:
                def emit_qk_one(p, which, wqk):
                    pqk = ps_a.tile([128, S], F32, tag="s2", bufs=2,
                                    name=f"pqk{which}{p}")
                    for n in range(2):
                        sl = slice(512 * n, 512 * (n + 1))
                        for ko in range(NCH):
                            nc.tensor.matmul(pqk[:, sl], wqk[:, ko, :],
                                             hT[ko][:, sl],
                                             start=(ko == 0),
                                             stop=(ko == NCH - 1))
                    t = arena.tile([128, S], BF16, tag=f"{which}T", bufs=2,
                                   name=f"{which}T{p}")
                    nc.vector.tensor_copy(out=t, in_=pqk)
                    return t

                def emit_qkT(p):
                    wq, wk = qk_w.pop(p)
                    return [emit_qk_one(p, "q", wq), emit_qk_one(p, "k", wk)]

                def attend_pair(p, qTp, kTp, qk_hook=None):
                    # heads A=2p (partitions 0:64) and B=2p+1 (64:128)
                    # interleaved kc-by-kc; the two heads' score matmuls
                    # target disjoint PE row groups and run concurrently.
                    po = [ps_a.tile([128, S], F32, tag="po", bufs=2,
                                    name=f"po{2 * p + i}") for i in (0, 1)]
                    for kc in range(NT):
                        if qk_hook is not None:
                            qk_hook(kc)
                        kcs = slice(128 * kc, 128 * (kc + 1))
                        pS = [ps_a.tile([128, S], F32, tag="s2", bufs=2,
                                        name=f"pS{2 * p + i}_{kc}")
                              for i in (0, 1)]
                        for n in range(2):
                            sl = slice(512 * n, 512 * (n + 1))
                            for i in (0, 1):
                                base = 64 * i
                                nc.tensor.matmul(pS[i][:, sl],
                                                 kTp[base:base + 64, kcs],
                                                 qTp[base:base + 64, sl])
                        expSs = []
                        for i in (0, 1):
                            expS = work.tile([128, S], BF16, tag="expS",
                                             bufs=3, name=f"expS{2 * p + i}_{kc}")
                            nc.scalar.activation(out=expS, in_=pS[i],
                                                 func=AF.Exp,
                                                 bias=0.0, scale=ATT_SCALE)
                            expSs.append(expS)
                        for i in (0, 1):
                            for n in range(2):
                                sl = slice(512 * n, 512 * (n + 1))
                                nc.tensor.matmul(
                                    po[i][0:65, sl],
                                    v_aug[:, kc, 2 * p + i, 0:65],
                                    expSs[i][:, sl],
                                    start=(kc == 0), stop=(kc == NT - 1))
                    return po

                def normalize(hh, po):
                    # copy [65,S] out of PSUM (frees po), reciprocal of the
                    # denominator row in place, 64-row broadcast on gpsimd,
                    # then one DVE mul into attnT.
                    sc = work.tile([65, S], F32, tag="bigst", bufs=2,
                                   name=f"sc{hh}")
                    nc.vector.tensor_copy(out=sc, in_=po[0:65, :])
                    r_raw = work.tile([1, S], F32, tag="mid", bufs=3,
                                      name=f"r_raw{hh}")
                    nc.sync.dma_start(out=r_raw, in_=sc[64:65, :])
                    r_rec = work.tile([1, S], F32, tag="mid", bufs=3,
                                      name=f"r_rec{hh}")
                    nc.vector.reciprocal_approx_fast(out=r_rec, in_=r_raw)
                    c2 = hh // 2
                    osc = None
                    if hh % 2 == 1:
                        osc = work.tile([64, S], F32R, tag="mid", bufs=3,
                                        name=f"osc{hh}")
                    for nn in range(2):
                        sl = slice(512 * nn, 512 * (nn + 1))
                        pr = work.tile([64, 512], F32, tag="pr", bufs=2,
                                       name=f"pr{hh}_{nn}")
                        nc.gpsimd.partition_broadcast(pr, r_rec[:, sl])
                        dst = attnT[c2][0:64, sl] if hh % 2 == 0 else osc[:, sl]
                        nc.vector.tensor_mul(out=dst, in0=sc[0:64, sl], in1=pr)
                    if hh % 2 == 1:
                        nc.gpsimd.dma_start(out=attnT[c2][64:128, :],
                                            in_=osc)

                qkT_next = emit_qkT(0)
                for p in range(6):
                    qTp, kTp = qkT_next
                    if p + 2 < 6:
                        qk_w[p + 2] = dma_qk(p + 2)
                    nxt = {}

                    def qk_hook(kc, p=p, nxt=nxt):
                        if p + 1 >= 6:
                            return
                        if kc == 2:
                            wq, wk = qk_w.pop(p + 1)
                            nxt["wk"] = wk
                            nxt["q"] = emit_qk_one(p + 1, "q", wq)
                        elif kc == 5:
                            nxt["k"] = emit_qk_one(p + 1, "k", nxt.pop("wk"))

                    po01 = attend_pair(p, qTp, kTp, qk_hook)
                    if p + 1 < 6:
                        qkT_next = [nxt["q"], nxt["k"]]
                    normalize(2 * p, po01[0])
                    if p == 3:
                        # prefetch proj weights during the 5th attention pair
                        for mc in range(NCH):
                            wp = wpp.tile([128, NCH, 128], F32R, tag=f"wp{mc}",
                                          bufs=1, name=f"wp{mc}")
                            nc.scalar.dma_start(
                                out=wp,
                                in_=w_proj_d.ap()[:, 128 * mc:128 * (mc + 1)]
                                .rearrange("(ko ki) m -> ki ko m", ki=128)
                                .bitcast(F32R))
                            wp_tiles.append(wp)
                    normalize(2 * p + 1, po01[1])

            # ------------- proj + residual + LN2 (wp still live) -------------
        # wvqk closed: v_aug / qk weights SBUF reclaimed. fc1 weights start
        # streaming here, while proj/LN2 (wpp still open) run.
        wm1 = ctx.enter_context(tc.tile_pool(name="wm1", bufs=1))
        w1_tiles = {}

        def dma_w1(mc):
            w1 = wm1.tile([128, NCH, 128], F32R, tag="w1", bufs=6,
                          name=f"w1_{mc}")
            nc.sync.dma_start(
                out=w1,
                in_=w_fc1_d.ap()[:, 128 * mc:128 * (mc + 1)]
                .rearrange("(ko ki) m -> ki ko m", ki=128)
                .bitcast(F32R))
            w1_tiles[mc] = w1

        for mc in range(6):
            dma_w1(mc)

        ps_m = ctx.enter_context(
            tc.tile_pool(name="ps_m", bufs=1, space="PSUM"))

        def proj_half(n):
            sl = slice(512 * n, 512 * (n + 1))
            for mc in range(NCH):
                py = ps_m.tile([128, 512], F32, tag="b1", bufs=6,
                               name=f"py{mc}_{n}")
                for ko in range(NCH):
                    nc.tensor.matmul(py, wp_tiles[mc][:, ko, :],
                                     attnT[ko][:, sl],
                                     start=(ko == 0), stop=(ko == NCH - 1))
                nc.vector.scalar_tensor_tensor(
                    out=out1T[mc][:, sl], in0=py,
                    scalar=bpc[:, mc:mc + 1],
                    in1=xT[mc].bitcast(F32)[:, sl],
                    op0=ALU.add, op1=ALU.add)

        proj_half(0)
        s20 = ln_stats_half(ps_m, out1T, 0, "l2")
        proj_half(1)
        rstd20, negmuR20 = ln_chain_half(s20, 0, "l2")
        s21 = ln_stats_half(ps_m, out1T, 1, "l2")
        A20, B20 = ln_bcast_half(ps_m, rstd20, negmuR20, "l2", 0,
                                 tag="b1", bufs=6)
        rstd21, negmuR21 = ln_chain_half(s21, 1, "l2")
        h2T = hT  # reuse (hT dead after the last qk^T matmuls)
        ln_apply_half(out1T, h2T, A20, B20, g2c, b2c, 0, "l2")
        jps2 = ps_m.tile([1, 128], F32, tag="b1", bufs=6, name="warm_ps2")
        for _ in range(60):
            nc.tensor.matmul(jps2, ones_col, warm)
        A21, B21 = ln_bcast_half(ps_m, rstd21, negmuR21, "l2", 1,
                                 tag="b1", bufs=6)
        ln_apply_half(out1T, h2T, A21, B21, g2c, b2c, 1, "l2")
        for _ in range(40):
            nc.tensor.matmul(jps2, ones_col, warm)
        es_wp.close()  # proj weights dead

        with tc.tile_pool(name="wmlp", bufs=1) as wmlp:
            if True:
                if dbg:
                    for c in range(NCH):
                        nc.sync.dma_start(
                            out=dbg_attn.ap()[128 * c:128 * (c + 1), :],
                            in_=attnT[c].bitcast(F32))
                        nc.sync.dma_start(
                            out=dbg_out1.ap()[128 * c:128 * (c + 1), :],
                            in_=out1T[c].bitcast(F32))
                        nc.sync.dma_start(
                            out=dbg_h2.ap()[128 * c:128 * (c + 1), :],
                            in_=h2T[c].bitcast(F32))

                # ---------------- fc1 + gelu ----------------
                # a1 tile j ([128, 2048] bf16) holds hidden chunks 2j / 2j+1,
                # aliased onto xT (dead after proj residual) and attnT (dead
                # after proj matmuls).
                a1 = []
                for j in range(12):
                    tag = f"xT{j}" if j < 6 else f"attnT{j - 6}"
                    a1.append(arena.tile([128, 2 * S], BF16, tag=tag,
                                         name=f"a1_{j}"))

                w2f_tiles = {}
                w2_tiles = {}

                def dma_w2(mc, half):
                    w2f = wmlp.tile([128, NFH // 2, 128], F32, tag="w2f",
                                    bufs=2, name=f"w2f_{mc}_{half}")
                    nc.scalar.dma_start(
                        out=w2f,
                        in_=w_fc2_d.ap()[1536 * half:1536 * (half + 1),
                                         128 * mc:128 * (mc + 1)]
                        .rearrange("(ko ki) m -> ki ko m", ki=128))
                    w2f_tiles[(mc, half)] = w2f

                def cast_w2(mc):
                    for half in (0, 1):
                        w2 = wmlp.tile([128, NFH // 2, 128], BF16, tag="w2",
                                       bufs=4, name=f"w2_{mc}_{half}")
                        nc.scalar.copy(out=w2, in_=w2f_tiles.pop((mc, half)))
                        w2_tiles[(mc, half)] = w2

                for mc in range(NFH):
                    if mc + 6 < NFH:
                        dma_w1(mc + 6)
                    elif mc == NFH - 3:
                        dma_w2(0, 0)
                        dma_w2(0, 1)
                    elif mc == NFH - 2:
                        dma_w2(1, 0)
                        dma_w2(1, 1)
                    elif mc == NFH - 1:
                        cast_w2(0)
                        cast_w2(1)
                    w1 = w1_tiles.pop(mc)
                    dst = a1[mc // 2][:, S * (mc % 2):S * (mc % 2) + S]
                    for n in range(2):
                        sl = slice(512 * n, 512 * (n + 1))
                        pg = ps_m.tile([128, 512], F32, tag="b1", bufs=6,
                                       name=f"pg{mc}_{n}")
                        for ko in range(NCH):
                            nc.tensor.matmul(pg, w1[:, ko, :], h2T[ko][:, sl],
                                             start=(ko == 0),
                                             stop=(ko == NCH - 1))
                        nc.scalar.activation(out=dst[:, sl], in_=pg,
                                             func=AF.Gelu,
                                             bias=bf1[:, mc:mc + 1], scale=1.0)

                # ---------------- fc2 + residual + transpose out ----------
                for mc in range(NCH):
                    if mc + 2 < NCH:
                        dma_w2(mc + 2, 0)
                        dma_w2(mc + 2, 1)
                    w2a = w2_tiles.pop((mc, 0))
                    w2b = w2_tiles.pop((mc, 1))
                    pending_cast = mc + 2 if mc + 2 < NCH else None
                    o_fin = work.tile([128, S], BF16, tag="mid", bufs=3,
                                      name=f"ofin{mc}")
                    for n in range(2):
                        sl = slice(512 * n, 512 * (n + 1))
                        py2 = ps_m.tile([128, 512], F32, tag="b1", bufs=6,
                                        name=f"py2_{mc}_{n}")
                        for f in range(NFH):
                            wt = w2a if f < 12 else w2b
                            rhs = a1[f // 2][:, S * (f % 2) + 512 * n:
                                             S * (f % 2) + 512 * (n + 1)]
                            nc.tensor.matmul(py2, wt[:, f % 12, :], rhs,
                                             start=(f == 0),
                                             stop=(f == NFH - 1))
                            if n == 0 and f == 12 and pending_cast is not None:
                                cast_w2(pending_cast)
                                pending_cast = None
                        nc.vector.scalar_tensor_tensor(
                            out=o_fin[:, sl], in0=py2,
                            scalar=bf2c[:, mc:mc + 1],
                            in1=out1T[mc].bitcast(F32)[:, sl],
                            op0=ALU.add, op1=ALU.add)
                    # transpose this channel chunk back (bf16, single-pass)
                    # and store its column block of the output.
                    ostage = work.tile([128, NT, 128], F32, tag="bigst",
                                       bufs=2, name=f"ostage{mc}")
                    for a in range(NT):
                        pst = ps_m.tile([128, 128], BF16, tag="b1", bufs=6,
                                        name=f"pto{mc}_{a}")
                        nc.tensor.transpose(
                            pst, o_fin[:, 128 * a:128 * (a + 1)], ident_bf)
                        nc.vector.tensor_copy(out=ostage[:, a, :], in_=pst)
                    nc.sync.dma_start(
                        out=out_d.ap()[:, 128 * mc:128 * (mc + 1)]
                        .rearrange("(a p) m -> p a m", p=128),
                        in_=ostage)

    nc.compile()
    return nc


def _get_nc():
    if "nc" not in _cached:
        _cached["nc"] = build()
    return _cached["nc"]


def kernel(**inputs):
    nc = _get_nc()
    x = np.ascontiguousarray(np.asarray(inputs["x"], dtype=np.float32))
    weights = {
        k: np.ascontiguousarray(np.asarray(inputs[k], dtype=np.float32))
        for k in ("ln1_g", "ln1_b", "w_qkv", "w_proj", "b_proj",
                  "ln2_g", "ln2_b", "w_fc1", "b_fc1", "w_fc2", "b_fc2")
    }
    in_maps = [{"x": x[i], **weights} for i in range(N_CORES)]
    trace = bool(int(os.environ.get("BASS_KERNEL_TRACE", "0")))
    res = run_bass_kernel_spmd(nc, in_maps, list(range(N_CORES)), trace=trace)
    _cached["last_exec_time_ns"] = res.exec_time_ns
    out = np.stack([res.results[i]["out"] for i in range(N_CORES)], axis=0)
    return out.astype(np.float32)


# revision 43
# speedup vs baseline: 1.2137x; 1.2137x over previous
"""Trainium2 Bass kernel for a pre-norm transformer block (nn_Block_38843684225792).

Full inputs -> full outputs. Sharding: data-parallel over batch, one batch
element per NeuronCore (8 cores). Inside each core the block is computed
channel-major (channels on SBUF partitions) so every matmul contracts over
the partition dim; x is transposed once on entry and the result transposed
back on exit via PE transposes.

v2: all weights DMA'd directly from their natural DRAM layout (no
DRAM->DRAM reorg), biases folded into PSUM accumulation via K=1 outer
products, LayerNorm scale/shift broadcast via K=1 PE matmuls + DVE apply
(no scalar-engine ACTs on the LN path), softmax denominators normalized
with a 64-row gpsimd broadcast, fc2 uses f32r stationary weights with bf16
moving activations (no weight casts), and output transposes stream out in
bf16 interleaved with the fc2 chains.

Shapes (per core): x [1024, 768], heads=12, hd=64, mlp hidden=3072.
"""

import os
import sys

sys.path.insert(0, "/opt/trn_rl_repo")

import numpy as np

import concourse.bass as bass
import concourse.tile as tile
from concourse import bacc, mybir
from concourse.bass_utils import run_bass_kernel_spmd
from concourse.masks import make_identity

F32 = mybir.dt.float32
F32R = mybir.dt.float32r
BF16 = mybir.dt.bfloat16
AF = mybir.ActivationFunctionType
ALU = mybir.AluOpType

N_CORES = 8
S = 1024          # sequence length per core
C = 768           # model dim
H = 12            # heads
HD = 64           # head dim
HID = 3072        # mlp hidden
NCH = C // 128    # 6 channel chunks
NT = S // 128     # 8 token chunks
NFH = HID // 128  # 24 hidden chunks
EPS = 1e-5
ATT_SCALE = HD ** -0.5  # 0.125

_cached = {}


def build():
    nc = bacc.Bacc(None, target_bir_lowering=False, debug=False)
    x_d = nc.declare_dram_parameter("x", [S, C], F32, isOutput=False)
    ln1_g_d = nc.declare_dram_parameter("ln1_g", [C], F32, isOutput=False)
    ln1_b_d = nc.declare_dram_parameter("ln1_b", [C], F32, isOutput=False)
    w_qkv_d = nc.declare_dram_parameter("w_qkv", [C, 3 * C], F32, isOutput=False)
    w_proj_d = nc.declare_dram_parameter("w_proj", [C, C], F32, isOutput=False)
    b_proj_d = nc.declare_dram_parameter("b_proj", [C], F32, isOutput=False)
    ln2_g_d = nc.declare_dram_parameter("ln2_g", [C], F32, isOutput=False)
    ln2_b_d = nc.declare_dram_parameter("ln2_b", [C], F32, isOutput=False)
    w_fc1_d = nc.declare_dram_parameter("w_fc1", [C, HID], F32, isOutput=False)
    b_fc1_d = nc.declare_dram_parameter("b_fc1", [HID], F32, isOutput=False)
    w_fc2_d = nc.declare_dram_parameter("w_fc2", [HID, C], F32, isOutput=False)
    b_fc2_d = nc.declare_dram_parameter("b_fc2", [C], F32, isOutput=False)
    out_d = nc.declare_dram_parameter("out", [S, C], F32, isOutput=True)
    dbg = bool(int(os.environ.get("BASS_DEBUG_TAPS", "0")))
    if dbg:
        dbg_h = nc.declare_dram_parameter("dbg_h", [C, S], F32, isOutput=True)
        dbg_attn = nc.declare_dram_parameter("dbg_attn", [C, S], F32, isOutput=True)
        dbg_out1 = nc.declare_dram_parameter("dbg_out1", [C, S], F32, isOutput=True)
        dbg_h2 = nc.declare_dram_parameter("dbg_h2", [C, S], F32, isOutput=True)

    from contextlib import ExitStack
    with tile.TileContext(nc) as tc, ExitStack() as ctx:
        consts = ctx.enter_context(tc.tile_pool(name="consts", bufs=1))
        arena = ctx.enter_context(tc.tile_pool(name="arena", bufs=1))
        work = ctx.enter_context(tc.tile_pool(name="work", bufs=1))

        # ---------------- constants ----------------
        ident = consts.tile([128, 128], F32, name="ident")
        make_identity(nc, ident)
        ident_bf = consts.tile([128, 128], BF16, name="ident_bf")
        nc.vector.tensor_copy(out=ident_bf, in_=ident)
        ones_row = consts.tile([1, 512], F32, name="ones_row")
        nc.vector.memset(ones_row, 1.0)
        ones_row_r = consts.tile([1, 512], F32R, name="ones_row_r")
        nc.vector.tensor_copy(out=ones_row_r, in_=ones_row)
        eps_ap = consts.tile([1, 1], F32, name="eps_ap")
        nc.vector.memset(eps_ap, EPS)
        eps_col = consts.tile([128, 1], F32, name="eps_col")
        nc.vector.memset(eps_col, EPS)
        ones_f32 = consts.tile([128, 1], F32, name="ones_f32")
        nc.vector.memset(ones_f32, 1.0)
        ones_col = consts.tile([128, 1], F32R, name="ones_col")
        nc.vector.tensor_copy(out=ones_col, in_=ones_f32)  # stat sums
        ones_r128 = consts.tile([1, 128], F32, name="ones_r128")
        nc.vector.memset(ones_r128, 1.0)
        ones_col_row = consts.tile([1, 128], F32R, name="ones_col_row")
        nc.vector.tensor_copy(out=ones_col_row, in_=ones_r128)  # K=1 bcasts

        # per-channel vectors, [128, n] column layout
        def load_chanvec(dram_t, name, width):
            t = consts.tile([128, width], F32, name=name)
            nc.sync.dma_start(out=t, in_=dram_t.ap().rearrange("(o p) -> p o", p=128))
            return t

        g1c = load_chanvec(ln1_g_d, "g1c", NCH)
        b1c = load_chanvec(ln1_b_d, "b1c", NCH)
        g2c = load_chanvec(ln2_g_d, "g2c", NCH)
        b2c = load_chanvec(ln2_b_d, "b2c", NCH)
        bf1 = load_chanvec(b_fc1_d, "bf1", NFH)

        # bias row vectors [1, C] for K=1 outer-product bias folds
        def load_rowvec(dram_t, name, width):
            t = consts.tile([1, width], F32R, name=name)
            nc.sync.dma_start(
                out=t,
                in_=dram_t.ap().rearrange("(a c) -> a c", a=1).bitcast(F32R))
            return t

        bpc = load_chanvec(b_proj_d, "bpc", NCH)
        bf2c = load_chanvec(b_fc2_d, "bf2c", NCH)

        # ---------------- persistent activation tiles ----------------
        xT = [arena.tile([128, S], F32R, tag=f"xT{c}", name=f"xT{c}")
              for c in range(NCH)]
        hT = [arena.tile([128, S], F32R, tag=f"hT{c}", name=f"hT{c}")
              for c in range(NCH)]
        attnT = [arena.tile([128, S], F32R, tag=f"attnT{c}", name=f"attnT{c}")
                 for c in range(NCH)]
        out1T = [arena.tile([128, S], F32R, tag=f"out1T{c}", name=f"out1T{c}")
                 for c in range(NCH)]

        # ================= LayerNorm helper pieces =================
        def ln_stats_half(ps, src_tiles, n, tagp):
            # psum2[1,0:512] = sum(x), [1,512:1024] = sum(x^2), one token half
            sl = slice(512 * n, 512 * (n + 1))
            psum2 = ps.tile([1, S], F32, tag="stat", bufs=1,
                            name=f"ln_s{tagp}{n}")
            for c in range(NCH):
                x2 = work.tile([128, 512], F32R, tag="mid", bufs=3,
                               name=f"ln_x2_{tagp}{n}{c}")
                nc.vector.tensor_mul(out=x2, in0=src_tiles[c].bitcast(F32)[:, sl],
                                     in1=src_tiles[c].bitcast(F32)[:, sl])
                nc.tensor.matmul(psum2[:, 0:512], ones_col, src_tiles[c][:, sl],
                                 start=(c == 0), stop=(c == NCH - 1))
                nc.tensor.matmul(psum2[:, 512:1024], ones_col, x2,
                                 start=(c == 0), stop=(c == NCH - 1))
            return psum2

        def ln_chain_half(psum2, n, tagp):
            mu = work.tile([1, 512], F32, tag="stats", bufs=4,
                           name=f"ln_mu{tagp}{n}")
            nc.scalar.mul(out=mu, in_=psum2[:, 0:512], mul=1.0 / C)
            ex2 = work.tile([1, 512], F32, tag="stats", bufs=4,
                            name=f"ln_ex2{tagp}{n}")
            nc.scalar.mul(out=ex2, in_=psum2[:, 512:1024], mul=1.0 / C)
            var = work.tile([1, 512], F32, tag="stats", bufs=4,
                            name=f"ln_var{tagp}{n}")
            nc.vector.tensor_mul(out=var, in0=mu, in1=mu)
            nc.vector.tensor_sub(out=var, in0=ex2, in1=var)
            nc.scalar.activation(out=var, in_=var, func=AF.Ln, bias=eps_ap,
                                 scale=1.0)
            rstd = work.tile([1, 512], F32R, tag="stats", bufs=4,
                             name=f"ln_rstd{tagp}{n}")
            nc.scalar.activation(out=rstd, in_=var, func=AF.Exp, bias=0.0,
                                 scale=-0.5)
            negmuR = work.tile([1, 512], F32R, tag="stats", bufs=4,
                               name=f"ln_nmu{tagp}{n}")
            nc.vector.scalar_tensor_tensor(out=negmuR, in0=mu, scalar=-1.0,
                                           in1=rstd.bitcast(F32), op0=ALU.mult,
                                           op1=ALU.mult)
            return rstd, negmuR

        def ln_bcast_half(ps, rstd, negmuR, tagp, n, tag="bc", bufs=2):
            # A0 = ones ⊗ rstd, B0 = ones ⊗ (-mu*rstd)   (both [128, 512])
            A0 = ps.tile([128, 512], F32, tag=tag, bufs=bufs,
                         name=f"ln_A{tagp}{n}")
            nc.tensor.matmul(A0, ones_col_row, rstd.bitcast(F32R))
            B0 = ps.tile([128, 512], F32, tag=tag, bufs=bufs,
                         name=f"ln_B{tagp}{n}")
            nc.tensor.matmul(B0, ones_col_row, negmuR.bitcast(F32R))
            return A0, B0

        def ln_apply_half(src_tiles, dst_tiles, A0, B0, gc, bc, n, tagp):
            sl = slice(512 * n, 512 * (n + 1))
            # SBUF copies of the broadcasts so chunks 4-5 can run on gpsimd
            # (which cannot read PSUM) in parallel with the DVE chunks.
            A0s = work.tile([128, 512], F32, tag="gtmp", bufs=2,
                            name=f"ln_A0s{tagp}{n}")
            nc.vector.tensor_copy(out=A0s, in_=A0)
            B0s = work.tile([128, 512], F32, tag="gtmp", bufs=2,
                            name=f"ln_B0s{tagp}{n}")
            nc.vector.tensor_copy(out=B0s, in_=B0)
            for c in range(NCH):
                eng = nc.gpsimd if c >= 4 else nc.vector
                a0, b0 = (A0s, B0s) if c >= 4 else (A0, B0)
                tg = "gtmp2" if c >= 4 else "tmp"
                t = work.tile([128, 512], F32, tag=tg, bufs=2,
                              name=f"ln_t{tagp}{n}{c}")
                eng.tensor_mul(out=t, in0=src_tiles[c].bitcast(F32)[:, sl],
                               in1=a0)
                eng.tensor_add(out=t, in0=t, in1=b0)
                eng.tensor_scalar(
                    out=dst_tiles[c][:, sl], in0=t,
                    scalar1=gc[:, c:c + 1], scalar2=bc[:, c:c + 1],
                    op0=ALU.mult, op1=ALU.add)

        # ---------------- attention-side weights pool ----------------
        with tc.tile_pool(name="wvqk", bufs=1) as wvqk:
            def dma_qk(p):
                tiles = []
                for which, m in (("q", p), ("k", 6 + p)):
                    wqk = wvqk.tile([128, NCH, 128], F32R, tag=f"w{which}",
                                    bufs=2, name=f"w{which}{p}")
                    nc.scalar.dma_start(
                        out=wqk,
                        in_=w_qkv_d.ap()[:, 128 * m:128 * (m + 1)]
                        .rearrange("(ko ki) m -> ki ko m", ki=128)
                        .bitcast(F32R))
                    tiles.append(wqk)
                return tiles

            # v_aug[p, mt, head, 66]: [v(64), one, pad]; the ones column makes
            # the O matmul emit softmax denominators at psum partition 64.
            v_aug = wvqk.tile([128, NT, H, 66], BF16, tag="v_aug", name="v_aug")
            nc.vector.memset(v_aug[:, :, :, 64:65], 1.0)

            # ------- entry: v/qk weight DMAs, transposes, LN1, v stage ------
            with tc.tile_pool(name="wvp", bufs=1) as wvp:
                with tc.tile_pool(name="ps_e", bufs=1, space="PSUM") as ps_e:
                    # token-major LN1 stat columns: sums/sumsq per token
                    sumsT = work.tile([128, NT], F32, tag="stats", bufs=4,
                                      name="l1_sumsT")
                    sqT = work.tile([128, NT], F32, tag="stats", bufs=4,
                                    name="l1_sqT")

                    def transpose_in(a):
                        # 6-way DMA split: one 512B-per-partition strip per
                        # queue so x streams in at full aggregate bandwidth.
                        x_sb = work.tile([128, C], F32, tag="bigst", bufs=2,
                                         name=f"x_sb{a}")
                        for c in range(NCH):
                            nc.sync.dma_start(
                                out=x_sb[:, 128 * c:128 * (c + 1)],
                                in_=x_d.ap()[128 * a:128 * (a + 1),
                                             128 * c:128 * (c + 1)])
                        # free-dim reductions on the scalar engine give this
                        # chunk's LN1 sums without touching the PE
                        jnk = work.tile([128, C], BF16, tag="actjunk", bufs=2,
                                        name=f"actjunk{a}")
                        nc.scalar.activation(out=jnk, in_=x_sb, func=AF.Copy,
                                             accum_out=sumsT[:, a:a + 1])
                        nc.scalar.activation(out=jnk, in_=x_sb, func=AF.Square,
                                             accum_out=sqT[:, a:a + 1])
                        for c in range(NCH):
                            pst = ps_e.tile([128, 128], F32, tag="small",
                                            bufs=3, name=f"ptx{a}_{c}")
                            nc.tensor.transpose(
                                pst, x_sb[:, 128 * c:128 * (c + 1)], ident)
                            nc.vector.tensor_copy(
                                out=xT[c][:, 128 * a:128 * (a + 1)], in_=pst)

                    # spin the PE for ~5us so HAM unthrottles to full
                    # clock before the first real transposes arrive.
                    warm = consts.tile([128, 128], F32R, name="warm")
                    nc.vector.tensor_copy(out=warm, in_=ident)
                    jps = ps_e.tile([1, 128], F32, tag="small", bufs=3,
                                    name="warm_ps")
                    for _ in range(45):
                        nc.tensor.matmul(jps, ones_col, warm)

                    transpose_in(0)
                    def spin(n):
                        for _ in range(n):
                            nc.tensor.matmul(jps, ones_col, warm)
                    spin(20)
                    transpose_in(1)
                    spin(20)
                    # weight streams queue behind the first two x chunks
                    wv = []
                    for i in range(12):
                        n, ko = i // NCH, i % NCH
                        w = wvp.tile([128, 384], F32R, tag=f"wv{i}", bufs=1,
                                     name=f"wv{n}_{ko}")
                        nc.scalar.dma_start(
                            out=w,
                            in_=w_qkv_d.ap()[128 * ko:128 * (ko + 1),
                                             1536 + 384 * n:1536 + 384 * (n + 1)]
                            .bitcast(F32R))
                        wv.append(w)
                    qk_w = {0: dma_qk(0), 1: dma_qk(1)}

                    def v_stage(mts):
                        for nn in range(2):  # halves of the 768 v-channels
                            for mt in mts:
                                pv = ps_e.tile([128, 384], F32, tag="small",
                                               bufs=3, name=f"pv{nn}_{mt}")
                                for ko in range(NCH):
                                    nc.tensor.matmul(
                                        pv, hT[ko][:, 128 * mt:128 * (mt + 1)],
                                        wv[nn * NCH + ko],
                                        start=(ko == 0), stop=(ko == NCH - 1))
                                pv3 = pv.rearrange("p (j d) -> p j d", d=HD)
                                nc.vector.tensor_copy(
                                    out=v_aug[:, mt, 6 * nn:6 * nn + 6, 0:64],
                                    in_=pv3)

                    transpose_in(2)
                    spin(20)
                    transpose_in(3)
                    spin(20)
                    for a in range(4, 8):
                        transpose_in(a)
                    # token-major chain: mu, var, rstd, -mu*rstd  ([128, NT])
                    muT = work.tile([128, NT], F32, tag="stats", bufs=4,
                                    name="l1_muT")
                    nc.scalar.mul(out=muT, in_=sumsT, mul=1.0 / C)
                    varT = work.tile([128, NT], F32, tag="stats", bufs=4,
                                     name="l1_varT")
                    nc.vector.tensor_mul(out=varT, in0=muT, in1=muT)
                    nc.vector.scalar_tensor_tensor(
                        out=varT, in0=sqT, scalar=1.0 / C, in1=varT,
                        op0=ALU.mult, op1=ALU.subtract)
                    nc.scalar.activation(out=varT, in_=varT, func=AF.Ln,
                                         bias=eps_col, scale=1.0)
                    rstdT = work.tile([128, NT], F32, tag="stats", bufs=4,
                                      name="l1_rstdT")
                    nc.scalar.activation(out=rstdT, in_=varT, func=AF.Exp,
                                         bias=0.0, scale=-0.5)
                    nmrT = work.tile([128, NT], F32, tag="stats", bufs=4,
                                     name="l1_nmrT")
                    nc.vector.scalar_tensor_tensor(
                        out=nmrT, in0=muT, scalar=-1.0, in1=rstdT,
                        op0=ALU.mult, op1=ALU.mult)
                    # pack [128, NT] token-major stats into [1, S] rows via a
                    # PE transpose + an SBUF-to-SBUF flattening DMA.
                    rows = {}
                    for nm, colT in (("rstd", rstdT), ("nmr", nmrT)):
                        pt = ps_e.tile([NT, 128], F32, tag="small", bufs=3,
                                       name=f"l1_{nm}_pt")
                        nc.tensor.transpose(pt, colT, ident)
                        r8 = work.tile([NT, 128], F32, tag="tmp", bufs=2,
                                       name=f"l1_{nm}_r8")
                        nc.vector.tensor_copy(out=r8, in_=pt)
                        row = work.tile([1, S], F32R, tag="mid", bufs=3,
                                        name=f"l1_{nm}_row")
                        nc.sync.dma_start(out=row.bitcast(F32), in_=r8)
                        rows[nm] = row
                    A00, B00 = ln_bcast_half(
                        ps_e, rows["rstd"].bitcast(F32)[:, 0:512],
                        rows["nmr"].bitcast(F32)[:, 0:512], "l1", 0)
                    ln_apply_half(xT, hT, A00, B00, g1c, b1c, 0, "l1")
                    spin(40)
                    A01, B01 = ln_bcast_half(
                        ps_e, rows["rstd"].bitcast(F32)[:, 512:1024],
                        rows["nmr"].bitcast(F32)[:, 512:1024], "l1", 1)
                    v_stage(range(0, 4))
                    ln_apply_half(xT, hT, A01, B01, g1c, b1c, 1, "l1")
                    v_stage(range(4, 8))

            if dbg:
                for c in range(NCH):
                    nc.sync.dma_start(
                        out=dbg_h.ap()[128 * c:128 * (c + 1), :],
                        in_=hT[c].bitcast(F32))

            # ---------------- attention ----------------
            wp_tiles = []
            es_wp = ExitStack()
            wpp = es_wp.enter_context(
                tc.tile_pool(name="wpp", bufs=1, side="right"))
            with tc.tile_pool(name="ps_a", bufs=1, space="PSUM") as ps_a:
                def emit_qk_one(p, which, wqk):
                    pqk = ps_a.tile([128, S], F32, tag="s2", bufs=2,
                                    name=f"pqk{which}{p}")
                    for n in range(2):
                        sl = slice(512 * n, 512 * (n + 1))
                        for ko in range(NCH):
                            nc.tensor.matmul(pqk[:, sl], wqk[:, ko, :],
                                             hT[ko][:, sl],
                                             start=(ko == 0),
                                             stop=(ko == NCH - 1))
                    t = arena.tile([128, S], BF16, tag=f"{which}T", bufs=2,
                                   name=f"{which}T{p}")
                    nc.vector.tensor_copy(out=t, in_=pqk)
                    return t

                def emit_qkT(p):
                    wq, wk = qk_w.pop(p)
                    return [emit_qk_one(p, "q", wq), emit_qk_one(p, "k", wk)]

                def attend_pair(p, qTp, kTp, qk_hook=None):
                    # heads A=2p (partitions 0:64) and B=2p+1 (64:128)
                    # interleaved kc-by-kc; the two heads' score matmuls
                    # target disjoint PE row groups and run concurrently.
                    po = [ps_a.tile([128, S], F32, tag="po", bufs=2,
                                    name=f"po{2 * p + i}") for i in (0, 1)]
                    for kc in range(NT):
                        if qk_hook is not None:
                            qk_hook(kc)
                        kcs = slice(128 * kc, 128 * (kc + 1))
                        pS = [ps_a.tile([128, S], F32, tag="s2", bufs=2,
                                        name=f"pS{2 * p + i}_{kc}")
                              for i in (0, 1)]
                        for n in range(2):
                            sl = slice(512 * n, 512 * (n + 1))
                            for i in (0, 1):
                                base = 64 * i
                                nc.tensor.matmul(pS[i][:, sl],
                                                 kTp[base:base + 64, kcs],
                                                 qTp[base:base + 64, sl])
                        expSs = []
                        for i in (0, 1):
                            expS = work.tile([128, S], BF16, tag="expS",
                                             bufs=3, name=f"expS{2 * p + i}_{kc}")
                            nc.scalar.activation(out=expS, in_=pS[i],
                                                 func=AF.Exp,
                                                 bias=0.0, scale=ATT_SCALE)
                            expSs.append(expS)
                        for i in (0, 1):
                            for n in range(2):
                                sl = slice(512 * n, 512 * (n + 1))
                                nc.tensor.matmul(
                                    po[i][0:65, sl],
                                    v_aug[:, kc, 2 * p + i, 0:65],
                                    expSs[i][:, sl],
                                    start=(kc == 0), stop=(kc == NT - 1))
                    return po

                def normalize(hh, po):
                    # copy [65,S] out of PSUM (frees po), reciprocal of the
                    # denominator row in place, 64-row broadcast on gpsimd,
                    # then one DVE mul into attnT.
                    sc = work.tile([65, S], F32, tag="bigst", bufs=2,
                                   name=f"sc{hh}")
                    nc.vector.tensor_copy(out=sc, in_=po[0:65, :])
                    r_raw = work.tile([1, S], F32, tag="mid", bufs=3,
                                      name=f"r_raw{hh}")
                    nc.sync.dma_start(out=r_raw, in_=sc[64:65, :])
                    r_rec = work.tile([1, S], F32, tag="mid", bufs=3,
                                      name=f"r_rec{hh}")
                    nc.vector.reciprocal_approx_fast(out=r_rec, in_=r_raw)
                    c2 = hh // 2
                    osc = None
                    if hh % 2 == 1:
                        osc = work.tile([64, S], F32R, tag="mid", bufs=3,
                                        name=f"osc{hh}")
                    for nn in range(2):
                        sl = slice(512 * nn, 512 * (nn + 1))
                        pr = work.tile([64, 512], F32, tag="pr", bufs=2,
                                       name=f"pr{hh}_{nn}")
                        nc.gpsimd.partition_broadcast(pr, r_rec[:, sl])
                        dst = attnT[c2][0:64, sl] if hh % 2 == 0 else osc[:, sl]
                        nc.vector.tensor_mul(out=dst, in0=sc[0:64, sl], in1=pr)
                    if hh % 2 == 1:
                        nc.gpsimd.dma_start(out=attnT[c2][64:128, :],
                                            in_=osc)

                qkT_next = emit_qkT(0)
                for p in range(6):
                    qTp, kTp = qkT_next
                    if p + 2 < 6:
                        qk_w[p + 2] = dma_qk(p + 2)
                    nxt = {}

                    def qk_hook(kc, p=p, nxt=nxt):
                        if p + 1 >= 6:
                            return
                        if kc == 2:
                            wq, wk = qk_w.pop(p + 1)
                            nxt["wk"] = wk
                            nxt["q"] = emit_qk_one(p + 1, "q", wq)
                        elif kc == 5:
                            nxt["k"] = emit_qk_one(p + 1, "k", nxt.pop("wk"))

                    po01 = attend_pair(p, qTp, kTp, qk_hook)
                    if p + 1 < 6:
                        qkT_next = [nxt["q"], nxt["k"]]
                    normalize(2 * p, po01[0])
                    if p == 3:
                        # prefetch proj weights during the 5th attention pair
                        for mc in range(NCH):
                            wp = wpp.tile([128, NCH, 128], F32R, tag=f"wp{mc}",
                                          bufs=1, name=f"wp{mc}")
                            nc.scalar.dma_start(
                                out=wp,
                                in_=w_proj_d.ap()[:, 128 * mc:128 * (mc + 1)]
                                .rearrange("(ko ki) m -> ki ko m", ki=128)
                                .bitcast(F32R))
                            wp_tiles.append(wp)
                    normalize(2 * p + 1, po01[1])

            # ------------- proj + residual + LN2 (wp still live) -------------
        # wvqk closed: v_aug / qk weights SBUF reclaimed. fc1 weights start
        # streaming here, while proj/LN2 (wpp still open) run.
        wm1 = ctx.enter_context(tc.tile_pool(name="wm1", bufs=1))
        w1_tiles = {}

        def dma_w1(mc):
            w1 = wm1.tile([128, NCH, 128], F32R, tag="w1", bufs=6,
                          name=f"w1_{mc}")
            nc.sync.dma_start(
                out=w1,
                in_=w_fc1_d.ap()[:, 128 * mc:128 * (mc + 1)]
                .rearrange("(ko ki) m -> ki ko m", ki=128)
                .bitcast(F32R))
            w1_tiles[mc] = w1

        for mc in range(6):
            dma_w1(mc)

        ps_m = ctx.enter_context(
            tc.tile_pool(name="ps_m", bufs=1, space="PSUM"))

        def proj_half(n):
            sl = slice(512 * n, 512 * (n + 1))
            for mc in range(NCH):
                py = ps_m.tile([128, 512], F32, tag="b1", bufs=6,
                               name=f"py{mc}_{n}")
                for ko in range(NCH):
                    nc.tensor.matmul(py, wp_tiles[mc][:, ko, :],
                                     attnT[ko][:, sl],
                                     start=(ko == 0), stop=(ko == NCH - 1))
                nc.vector.scalar_tensor_tensor(
                    out=out1T[mc][:, sl], in0=py,
                    scalar=bpc[:, mc:mc + 1],
                    in1=xT[mc].bitcast(F32)[:, sl],
                    op0=ALU.add, op1=ALU.add)

        proj_half(0)
        s20 = ln_stats_half(ps_m, out1T, 0, "l2")
        proj_half(1)
        rstd20, negmuR20 = ln_chain_half(s20, 0, "l2")
        s21 = ln_stats_half(ps_m, out1T, 1, "l2")
        A20, B20 = ln_bcast_half(ps_m, rstd20, negmuR20, "l2", 0,
                                 tag="b1", bufs=6)
        rstd21, negmuR21 = ln_chain_half(s21, 1, "l2")
        h2T = hT  # reuse (hT dead after the last qk^T matmuls)
        ln_apply_half(out1T, h2T, A20, B20, g2c, b2c, 0, "l2")
        jps2 = ps_m.tile([1, 128], F32, tag="b1", bufs=6, name="warm_ps2")
        for _ in range(40):
            nc.tensor.matmul(jps2, ones_col, warm)
        A21, B21 = ln_bcast_half(ps_m, rstd21, negmuR21, "l2", 1,
                                 tag="b1", bufs=6)
        ln_apply_half(out1T, h2T, A21, B21, g2c, b2c, 1, "l2")
        es_wp.close()  # proj weights dead

        with tc.tile_pool(name="wmlp", bufs=1) as wmlp:
            if True:
                if dbg:
                    for c in range(NCH):
                        nc.sync.dma_start(
                            out=dbg_attn.ap()[128 * c:128 * (c + 1), :],
                            in_=attnT[c].bitcast(F32))
                        nc.sync.dma_start(
                            out=dbg_out1.ap()[128 * c:128 * (c + 1), :],
                            in_=out1T[c].bitcast(F32))
                        nc.sync.dma_start(
                            out=dbg_h2.ap()[128 * c:128 * (c + 1), :],
                            in_=h2T[c].bitcast(F32))

                # ---------------- fc1 + gelu ----------------
                # a1 tile j ([128, 2048] bf16) holds hidden chunks 2j / 2j+1,
                # aliased onto xT (dead after proj residual) and attnT (dead
                # after proj matmuls).
                a1 = []
                for j in range(12):
                    tag = f"xT{j}" if j < 6 else f"attnT{j - 6}"
                    a1.append(arena.tile([128, 2 * S], BF16, tag=tag,
                                         name=f"a1_{j}"))

                w2f_tiles = {}
                w2_tiles = {}

                def dma_w2(mc, half):
                    w2f = wmlp.tile([128, NFH // 2, 128], F32, tag="w2f",
                                    bufs=2, name=f"w2f_{mc}_{half}")
                    nc.scalar.dma_start(
                        out=w2f,
                        in_=w_fc2_d.ap()[1536 * half:1536 * (half + 1),
                                         128 * mc:128 * (mc + 1)]
                        .rearrange("(ko ki) m -> ki ko m", ki=128))
                    w2f_tiles[(mc, half)] = w2f

                def cast_w2(mc):
                    for half in (0, 1):
                        w2 = wmlp.tile([128, NFH // 2, 128], BF16, tag="w2",
                                       bufs=4, name=f"w2_{mc}_{half}")
                        nc.scalar.copy(out=w2, in_=w2f_tiles.pop((mc, half)))
                        w2_tiles[(mc, half)] = w2

                for mc in range(NFH):
                    if mc + 6 < NFH:
                        dma_w1(mc + 6)
                    elif mc == NFH - 3:
                        dma_w2(0, 0)
                        dma_w2(0, 1)
                    elif mc == NFH - 2:
                        dma_w2(1, 0)
                        dma_w2(1, 1)
                    elif mc == NFH - 1:
                        cast_w2(0)
                        cast_w2(1)
                    w1 = w1_tiles.pop(mc)
                    dst = a1[mc // 2][:, S * (mc % 2):S * (mc % 2) + S]
                    for n in range(2):
                        sl = slice(512 * n, 512 * (n + 1))
                        pg = ps_m.tile([128, 512], F32, tag="b1", bufs=6,
                                       name=f"pg{mc}_{n}")
                        for ko in range(NCH):
                            nc.tensor.matmul(pg, w1[:, ko, :], h2T[ko][:, sl],
                                             start=(ko == 0),
                                             stop=(ko == NCH - 1))
                        nc.scalar.activation(out=dst[:, sl], in_=pg,
                                             func=AF.Gelu,
                                             bias=bf1[:, mc:mc + 1], scale=1.0)

                # ---------------- fc2 + residual + transpose out ----------
                for mc in range(NCH):
                    if mc + 2 < NCH:
                        dma_w2(mc + 2, 0)
                        dma_w2(mc + 2, 1)
                    w2a = w2_tiles.pop((mc, 0))
                    w2b = w2_tiles.pop((mc, 1))
                    pending_cast = mc + 2 if mc + 2 < NCH else None
                    o_fin = work.tile([128, S], BF16, tag="mid", bufs=3,
                                      name=f"ofin{mc}")
                    for n in range(2):
                        sl = slice(512 * n, 512 * (n + 1))
                        py2 = ps_m.tile([128, 512], F32, tag="b1", bufs=6,
                                        name=f"py2_{mc}_{n}")
                        for f in range(NFH):
                            wt = w2a if f < 12 else w2b
                            rhs = a1[f // 2][:, S * (f % 2) + 512 * n:
                                             S * (f % 2) + 512 * (n + 1)]
                            nc.tensor.matmul(py2, wt[:, f % 12, :], rhs,
                                             start=(f == 0),
                                             stop=(f == NFH - 1))
                            if n == 0 and f == 12 and pending_cast is not None:
                                cast_w2(pending_cast)
                                pending_cast = None
                        nc.vector.scalar_tensor_tensor(
                            out=o_fin[:, sl], in0=py2,
                            scalar=bf2c[:, mc:mc + 1],
                            in1=out1T[mc].bitcast(F32)[:, sl],
                            op0=ALU.add, op1=ALU.add)
                    # transpose this channel chunk back (bf16, single-pass)
                    # and store its column block of the output.
                    ostage = work.tile([128, NT, 128], F32, tag="bigst",
                                       bufs=2, name=f"ostage{mc}")
                    for a in range(NT):
                        pst = ps_m.tile([128, 128], BF16, tag="b1", bufs=6,
                                        name=f"pto{mc}_{a}")
                        nc.tensor.transpose(
                            pst, o_fin[:, 128 * a:128 * (a + 1)], ident_bf)
                        nc.vector.tensor_copy(out=ostage[:, a, :], in_=pst)
                    nc.sync.dma_start(
                        out=out_d.ap()[:, 128 * mc:128 * (mc + 1)]
                        .rearrange("(a p) m -> p a m", p=128),
                        in_=ostage)

    nc.compile()
    return nc


def _get_nc():
    if "nc" not in _cached:
        _cached["nc"] = build()
    return _cached["nc"]


def kernel(**inputs):
    nc = _get_nc()
    x = np.ascontiguousarray(np.asarray(inputs["x"], dtype=np.float32))
    weights = {
        k: np.ascontiguousarray(np.asarray(inputs[k], dtype=np.float32))
        for k in ("ln1_g", "ln1_b", "w_qkv", "w_proj", "b_proj",
                  "ln2_g", "ln2_b", "w_fc1", "b_fc1", "w_fc2", "b_fc2")
    }
    in_maps = [{"x": x[i], **weights} for i in range(N_CORES)]
    trace = bool(int(os.environ.get("BASS_KERNEL_TRACE", "0")))
    res = run_bass_kernel_spmd(nc, in_maps, list(range(N_CORES)), trace=trace)
    _cached["last_exec_time_ns"] = res.exec_time_ns
    out = np.stack([res.results[i]["out"] for i in range(N_CORES)], axis=0)
    return out.astype(np.float32)
